# revision 1
# baseline (speedup 1.0000x reference)
"""Dense causal transformer attention block on 8 Trainium2 NeuronCores.

Problem: out = CausalAttention(RoPE(x@wq, x@wk), x@wv) @ wo
  x [2, 4096, 2048], 16 heads x 128 dim, fp32 I/O.

Sharding: tensor-parallel over heads. Core c owns heads {2c, 2c+1}:
  - computes qT/kT/vT ([head_dim, seq] layout) for its heads from the full
    (host-pre-transposed) xT, RoPE applied on-chip, V re-transposed to
    [seq, head_dim] on the PE (identity-matmul transpose),
  - runs causal attention in transposed form (scoresT = k @ qT so the
    softmax weights come out as the moving operand of the A@V matmul —
    no on-chip transpose of the probability matrix needed),
  - denominators via an all-ones [128,128] stationary matmul (comes out
    pre-broadcast across partitions),
  - computes its partial output projection o_local @ wo[rows of its heads].
Host sums the 8 partial outputs (the wo row-parallel all-reduce).

Compute dtype bf16 (PE 1 cycle/row), accumulation fp32 in PSUM.
"""
import sys

for _p in ("/opt/trn_rl_repo",):
    if _p not in sys.path:
        sys.path.insert(0, _p)

import numpy as np
import ml_dtypes
from contextlib import ExitStack

import concourse.bass as bass
import concourse.tile as tile
from concourse import bacc, mybir
from concourse import bass_utils

B, S, D = 2, 4096, 2048
H, DH = 16, 128
HALF = DH // 2
NC = 8
HPC = H // NC          # heads per core = 2
DOUT = HPC * DH        # 256 local proj width
ROPE_BASE = 10000.0
SCALE = 1.0 / float(np.sqrt(DH))
SQ = 512               # query tile (free dim of scoresT)
SKB = 128              # key block (partitions of scoresT)
KM = D // 128          # 16 contraction blocks
NSQ = S // SQ          # 8 query tiles per batch
BF = mybir.dt.bfloat16
F32 = mybir.dt.float32

_CACHED = {}


def _build():
    nc = bacc.Bacc("TRN2", target_bir_lowering=False, debug=False, num_devices=NC)

    xT = nc.dram_tensor("xT", [D, B * S], BF, kind="ExternalInput").ap()
    wq = nc.dram_tensor("wq", [D, DOUT], BF, kind="ExternalInput").ap()
    wk = nc.dram_tensor("wk", [D, DOUT], BF, kind="ExternalInput").ap()
    wv = nc.dram_tensor("wv", [D, DOUT], BF, kind="ExternalInput").ap()
    wo = nc.dram_tensor("wo", [DOUT, D], BF, kind="ExternalInput").ap()
    cosf = nc.dram_tensor("cosf", [DH, S], F32, kind="ExternalInput").ap()
    sins = nc.dram_tensor("sins", [DH, S], F32, kind="ExternalInput").ap()
    masks = nc.dram_tensor("masks", [SKB, 4 * SQ], BF, kind="ExternalInput").ap()
    ones = nc.dram_tensor("ones", [128, 128], BF, kind="ExternalInput").ap()
    ident = nc.dram_tensor("ident", [128, 128], BF, kind="ExternalInput").ap()
    outp = nc.dram_tensor("outp", [B * S, D], BF, kind="ExternalOutput").ap()

    with tile.TileContext(nc) as tc, ExitStack() as ctx:
        const = ctx.enter_context(tc.tile_pool(name="const", bufs=1))
        xpool = ctx.enter_context(tc.tile_pool(name="xpool", bufs=18))
        qkv = ctx.enter_context(tc.tile_pool(name="qkv", bufs=1))
        rope = ctx.enter_context(tc.tile_pool(name="rope", bufs=2))
        attn = ctx.enter_context(tc.tile_pool(name="attn", bufs=4))
        opool = ctx.enter_context(tc.tile_pool(name="opool", bufs=4))

        # ---- persistent constants -------------------------------------
        # Single packed tile per weight; DMA emitted inside the first tile
        # body (the Sync sequencer issues DMA instructions at ~0.6us each,
        # so emission order directly sets time-to-first-matmul).
        wq_sb = const.tile([128, KM * DOUT], BF, name="wq_sb")
        wk_sb = const.tile([128, KM * DOUT], BF, name="wk_sb")
        wv_sb = const.tile([128, KM * DOUT], BF, name="wv_sb")
        ones_sb = const.tile([128, 128], BF, name="ones_sb")
        nc.sync.dma_start(ones_sb[:], ones[:])
        id_sb = const.tile([128, 128], BF, name="id_sb")
        nc.sync.dma_start(id_sb[:], ident[:])
        # cos/sin/masks/wo are needed only after the first matmuls; their
        # DMAs are emitted inside the first tile's body so the t=0 x tiles
        # win the early DMA queue slots.
        cos_sb = const.tile([DH, S], F32, name="cos_sb")
        sin_sb = const.tile([DH, S], F32, name="sin_sb")  # rows 0-63 = -sin
        mask_sb = const.tile([SKB, 4 * SQ], BF, name="mask_sb")
        wo_sb = const.tile([128, HPC * D], BF, name="wo_sb")    # [p, h*2048+n]

        qT = [qkv.tile([128, S], BF, tag=f"qT{j}", name=f"qT{j}") for j in range(HPC)]
        kT = [qkv.tile([128, S], BF, tag=f"kT{j}", name=f"kT{j}") for j in range(HPC)]
        vsb = [qkv.tile([128, S], BF, tag=f"v{j}", name=f"v{j}") for j in range(HPC)]
        oT = [qkv.tile([128, S], BF, tag=f"oT{j}", name=f"oT{j}") for j in range(HPC)]

        with tc.tile_pool(name="psm", bufs=1, space="PSUM") as psm:
            # ---- fully merged per-t pipeline ---------------------------
            # One 8-bank PSUM pool shared by both batches:
            #   pqk (1 bank)   q then k accumulation, per head, sequential
            #   pv  (1 bank)   v accumulation + PE-transpose targets
            #   pscr(2x2 bank) attention score pairs [128,1024]
            #   po  (1 bank)   A@V accumulator
            #   pd  (1 bank)   denominator accumulator
            # Out-proj pf tiles share the pscr tag's slots.
            for b in range(B):
                for t in range(NSQ):
                    s0 = t * SQ
                    xbt = [xpool.tile([128, 8 * SQ], BF, tag="xb", bufs=3,
                                      name=f"xbt{hh}") for hh in range(2)]
                    for hh in range(2):
                        nc.sync.dma_start(
                            xbt[hh][:].rearrange("p (a n) -> p a n", n=SQ),
                            xT[hh * 1024:(hh + 1) * 1024,
                               b * S + s0: b * S + s0 + SQ]
                            .rearrange("(a p) n -> p a n", p=128))
                        if b == 0 and t == 0 and hh == 0:
                            nc.sync.dma_start(
                                wq_sb[:].rearrange("p (a n) -> p a n", n=DOUT),
                                wq.rearrange("(a p) n -> p a n", p=128))
                    if b == 0 and t == 0:
                        nc.sync.dma_start(
                            wk_sb[:].rearrange("p (a n) -> p a n", n=DOUT),
                            wk.rearrange("(a p) n -> p a n", p=128))
                        nc.sync.dma_start(
                            wv_sb[:].rearrange("p (a n) -> p a n", n=DOUT),
                            wv.rearrange("(a p) n -> p a n", p=128))
                        nc.sync.dma_start(cos_sb[:], cosf[:])
                        nc.sync.dma_start(sin_sb[:], sins[:])
                        nc.sync.dma_start(mask_sb[:], masks[:])
                        nc.sync.dma_start(
                            wo_sb[:].rearrange("p (a n) -> p a n", n=D),
                            wo.rearrange("(a p) n -> p a n", p=128))
                    # --- projections + RoPE, head by head ---------------
                    for j in range(HPC):
                        for w_sb, dstt in ((wq_sb, qT[j]), (wk_sb, kT[j])):
                            pp = psm.tile([128, SQ], F32, tag="pqk", name="pp")
                            for km in range(KM):
                                nc.tensor.matmul(
                                    pp[:],
                                    w_sb[:, km * DOUT + j * DH:
                                         km * DOUT + (j + 1) * DH],
                                    xbt[km // 8][:, (km % 8) * SQ:
                                                 (km % 8 + 1) * SQ],
                                    start=km == 0, stop=km == KM - 1)
                            rt = rope.tile([128, SQ], F32, tag="rot", name="rt")
                            nc.vector.tensor_mul(
                                rt[0:HALF, :], pp[HALF:128, :],
                                sin_sb[0:HALF, s0:s0 + SQ])
                            nc.vector.tensor_mul(
                                rt[HALF:128, :], pp[0:HALF, :],
                                sin_sb[HALF:128, s0:s0 + SQ])
                            m1 = rope.tile([128, SQ], F32, tag="m1", name="m1")
                            nc.vector.tensor_mul(m1[:], pp[:], cos_sb[:, s0:s0 + SQ])
                            nc.vector.tensor_add(dstt[:, s0:s0 + SQ], m1[:], rt[:])
                        pv = psm.tile([128, SQ], F32, tag="pv", name="pv")
                        for km in range(KM):
                            nc.tensor.matmul(
                                pv[:],
                                wv_sb[:, km * DOUT + j * DH:
                                      km * DOUT + (j + 1) * DH],
                                xbt[km // 8][:, (km % 8) * SQ:
                                             (km % 8 + 1) * SQ],
                                start=km == 0, stop=km == KM - 1)
                        vt = rope.tile([128, SQ], BF, tag="vt", name="vt")
                        nc.scalar.copy(vt[:], pv[:])
                        for sub in range(4):
                            ptr = psm.tile([128, 128], BF, tag="pv", name="ptr")
                            nc.tensor.transpose(
                                ptr[:], vt[:, sub * 128:(sub + 1) * 128], id_sb[:])
                            nc.vector.tensor_copy(
                                vsb[j][:, (4 * t + sub) * 128:(4 * t + sub + 1) * 128],
                                ptr[:])
                    # --- causal attention for this query tile -----------
                    for j in range(HPC):
                        nblk = 4 * t + 4
                        npair = nblk // 2
                        nquad = npair // 2
                        po = psm.tile([128, SQ], F32, tag="po", name="po")
                        pd = psm.tile([128, SQ], F32, tag="pd", name="pd")
                        prev_et = None
                        for p in range(npair):
                            pscr = psm.tile([128, 2 * SQ], F32, tag="pscr",
                                            bufs=2, name="pscr")
                            for h in range(2):
                                u = 2 * p + h
                                nc.tensor.matmul(
                                    pscr[:, h * SQ:(h + 1) * SQ],
                                    kT[j][:, u * SKB:(u + 1) * SKB],
                                    qT[j][:, s0:s0 + SQ], start=True, stop=True,
                                    skip_group_check=True)
                            et = attn.tile([128, 2 * SQ], BF, tag="et", bufs=4,
                                           name="et")
                            nc.scalar.activation(
                                et[:], pscr[:], mybir.ActivationFunctionType.Exp,
                                scale=SCALE)
                            if 2 * p >= 4 * t:  # pair on the diagonal band
                                r = 2 * p - 4 * t   # 0 or 2
                                nc.vector.tensor_mul(
                                    et[:], et[:],
                                    mask_sb[:, r * SQ:(r + 2) * SQ])
                            for h in range(2):
                                u = 2 * p + h
                                nc.tensor.matmul(
                                    po[:], vsb[j][:, u * 128:(u + 1) * 128],
                                    et[:, h * SQ:(h + 1) * SQ],
                                    start=u == 0, stop=u == nblk - 1)
                            if p % 2 == 1:
                                qi = p // 2
                                qs = attn.tile([128, 2 * SQ], BF, tag="qs",
                                               bufs=2, name="qs")
                                nc.vector.tensor_add(qs[:], prev_et[:], et[:])
                                qs2 = attn.tile([128, SQ], BF, tag="qs2",
                                                bufs=2, name="qs2")
                                nc.vector.tensor_add(
                                    qs2[:], qs[:, 0:SQ], qs[:, SQ:2 * SQ])
                                nc.tensor.matmul(
                                    pd[:], ones_sb[:], qs2[:],
                                    start=qi == 0, stop=qi == nquad - 1)
                            prev_et = et
                        rec = attn.tile([128, SQ], F32, tag="rec", bufs=2,
                                        name="rec")
                        nc.vector.reciprocal(rec[:], pd[:])
                        nc.vector.tensor_mul(oT[j][:, s0:s0 + SQ], po[:], rec[:])
                    # --- out-proj for the 4 seq blocks completed at t ----
                    # pf tiles borrow the pscr tag's 2-bank slots.
                    for m in range(4 * t, 4 * t + 4):
                        for n in range(D // 512):
                            pf = psm.tile([128, 512], F32, tag="pscr", bufs=2,
                                          name="pf")
                            for jj in range(HPC):
                                nc.tensor.matmul(
                                    pf[:], oT[jj][:, m * 128:(m + 1) * 128],
                                    wo_sb[:, jj * D + n * 512:
                                          jj * D + (n + 1) * 512],
                                    start=jj == 0, stop=jj == HPC - 1)
                            ob = opool.tile([128, 512], BF, tag="ob", name="ob")
                            if (m + n) % 2 == 0:
                                nc.vector.tensor_copy(ob[:], pf[:])
                            else:
                                nc.scalar.copy(ob[:], pf[:])
                            nc.sync.dma_start(
                                outp[b * S + m * 128: b * S + (m + 1) * 128,
                                     n * 512:(n + 1) * 512], ob[:])

    nc.compile()
    return nc


def _host_inputs(x, wq, wk, wv, wo, cos, sin):
    bf16 = ml_dtypes.bfloat16
    xT = np.ascontiguousarray(x.reshape(B * S, D).T).astype(bf16)

    cos = np.asarray(cos, dtype=np.float32)        # [S, 64]
    sin = np.asarray(sin, dtype=np.float32)
    cosf = np.ascontiguousarray(
        np.concatenate([cos, cos], axis=1).T)      # [128, S]
    sins = np.concatenate([-sin, sin], axis=1).T   # rows 0-63 negated
    sins = np.ascontiguousarray(sins)

    i = np.arange(SKB)[:, None]
    jj = np.arange(SQ)[None, :]
    masks = np.concatenate(
        [(i + r * SKB <= jj) for r in range(4)], axis=1).astype(bf16)
    ones = np.ones((128, 128), dtype=bf16)
    ident = np.eye(128, dtype=bf16)

    in_maps = []
    for c in range(NC):
        lo = c * DOUT
        in_maps.append({
            "xT": xT,
            "wq": np.ascontiguousarray(wq[:, lo:lo + DOUT]).astype(bf16),
            "wk": np.ascontiguousarray(wk[:, lo:lo + DOUT]).astype(bf16),
            "wv": np.ascontiguousarray(wv[:, lo:lo + DOUT]).astype(bf16),
            "wo": np.ascontiguousarray(wo[lo:lo + DOUT, :]).astype(bf16),
            "cosf": cosf,
            "sins": sins,
            "masks": masks,
            "ones": ones,
            "ident": ident,
        })
    return in_maps


def kernel(x, wq, wk, wv, wo, cos, sin, _trace=False, _tmpdir=None):
    if "nc" not in _CACHED:
        _CACHED["nc"] = _build()
    nc = _CACHED["nc"]
    in_maps = _host_inputs(
        np.asarray(x, dtype=np.float32), np.asarray(wq, dtype=np.float32),
        np.asarray(wk, dtype=np.float32), np.asarray(wv, dtype=np.float32),
        np.asarray(wo, dtype=np.float32), cos, sin)
    res = bass_utils.run_bass_kernel_spmd(
        nc, in_maps, core_ids=list(range(NC)), trace=_trace, tmpdir=_tmpdir)
    acc = np.zeros((B * S, D), dtype=np.float32)
    for c in range(NC):
        acc += res.results[c]["outp"].astype(np.float32)
    out = acc.reshape(B, S, D)
    if _trace:
        _CACHED["last_results"] = res
    return out



# revision 5
# speedup vs baseline: 1.2222x; 1.2222x over previous
"""Dense causal transformer attention block on 8 Trainium2 NeuronCores.

Problem: out = CausalAttention(RoPE(x@wq, x@wk), x@wv) @ wo
  x [2, 4096, 2048], 16 heads x 128 dim, fp32 I/O.

Sharding: tensor-parallel over heads. Core c owns heads {2c, 2c+1}:
  - computes qT/kT ([head_dim, seq] layout) for its heads from the
    host-packed xP (all DMAs are contiguous-row), RoPE applied on-chip in
    bf16 (one ScalarE PSUM->SBUF copy, then 2x-mode DVE ops),
  - V is projected directly in [seq, head_dim] layout by using the x tile
    as the matmul stationary operand (no PE transposes needed),
  - runs causal attention in transposed form (scoresT = k @ qT so the
    softmax weights come out as the moving operand of the A@V matmul),
    with partial-width A@V matmuls on the diagonal band,
  - denominators via an all-ones [128,128] stationary matmul on
    DVE-pre-summed exp tiles; reciprocal via the fast custom DVE op,
  - output projection for query tile t is deferred into tile t+1's
    attention phase (interleaved m-blocks) so the softmax-normalize chain
    never stalls the in-order PE queue.
Host sums the 8 partial outputs (the wo row-parallel all-reduce).

Compute dtype bf16 (PE 1 col/cycle), accumulation fp32 in PSUM.
"""
import sys

for _p in ("/opt/trn_rl_repo",):
    if _p not in sys.path:
        sys.path.insert(0, _p)

import numpy as np
import ml_dtypes
from contextlib import ExitStack

import concourse.bass as bass
import concourse.tile as tile
from concourse import bacc, mybir
from concourse import bass_utils

B, S, D = 2, 4096, 2048
H, DH = 16, 128
HALF = DH // 2
NC = 8
HPC = H // NC          # heads per core = 2
DOUT = HPC * DH        # 256 local proj width
ROPE_BASE = 10000.0
SCALE = 1.0 / float(np.sqrt(DH))
SQ = 512               # query tile (free dim of scoresT)
SKB = 128              # key block (partitions of scoresT)
KM = D // 128          # 16 contraction blocks
NSQ = S // SQ          # 8 query tiles per batch
BF = mybir.dt.bfloat16
F32 = mybir.dt.float32

_CACHED = {}


def _build():
    nc = bacc.Bacc("TRN2", target_bir_lowering=False, debug=False, num_devices=NC)

    # xP: [128, (hh, b*8+t, a*512+n)] so each (b,t,hh) x-tile DMA is a
    # contiguous [128, 4096] read. Weights pre-packed the same way.
    xP = nc.dram_tensor("xP", [128, 2 * B * NSQ * 8 * SQ], BF,
                        kind="ExternalInput").ap()
    wq = nc.dram_tensor("wq", [128, KM * DOUT], BF, kind="ExternalInput").ap()
    wk = nc.dram_tensor("wk", [128, KM * DOUT], BF, kind="ExternalInput").ap()
    wv = nc.dram_tensor("wv", [128, KM * DOUT], BF, kind="ExternalInput").ap()
    wo = nc.dram_tensor("wo", [128, HPC * D], BF, kind="ExternalInput").ap()
    cosf = nc.dram_tensor("cosf", [DH, S], BF, kind="ExternalInput").ap()
    sins = nc.dram_tensor("sins", [DH, S], BF, kind="ExternalInput").ap()
    masks = nc.dram_tensor("masks", [SKB, 4 * SQ], BF, kind="ExternalInput").ap()
    ones = nc.dram_tensor("ones", [128, 128], BF, kind="ExternalInput").ap()
    outp = nc.dram_tensor("outp", [B * S, D], BF, kind="ExternalOutput").ap()

    XTILE = 8 * SQ     # 4096 cols per (b,t,hh) x tile

    with tile.TileContext(nc) as tc, ExitStack() as ctx:
        const = ctx.enter_context(tc.tile_pool(name="const", bufs=1))
        xpool = ctx.enter_context(tc.tile_pool(name="xpool", bufs=1))
        qkv = ctx.enter_context(tc.tile_pool(name="qkv", bufs=1))
        rope = ctx.enter_context(tc.tile_pool(name="rope", bufs=2))
        attn = ctx.enter_context(tc.tile_pool(name="attn", bufs=4))
        opool = ctx.enter_context(tc.tile_pool(name="opool", bufs=2))

        # ---- persistent constants (DMAs emitted in priority order) -----
        wq_sb = const.tile([128, KM * DOUT], BF, name="wq_sb")
        wk_sb = const.tile([128, KM * DOUT], BF, name="wk_sb")
        wv_sb = const.tile([128, KM * DOUT], BF, name="wv_sb")
        ones_sb = const.tile([128, 128], BF, name="ones_sb")
        nc.sync.dma_start(ones_sb[:], ones[:])
        cos_sb = const.tile([DH, S], BF, name="cos_sb")
        sin_sb = const.tile([DH, S], BF, name="sin_sb")  # rows 64-127 = -sin
        mask_sb = const.tile([SKB, 4 * SQ], BF, name="mask_sb")
        wo_sb = const.tile([128, HPC * D], BF, name="wo_sb")   # [p, jj*2048+n]

        qT = [qkv.tile([128, S], BF, tag=f"qT{j}", name=f"qT{j}") for j in range(HPC)]
        kT = [qkv.tile([128, S], BF, tag=f"kT{j}", name=f"kT{j}") for j in range(HPC)]
        # vsb: [seq-block u][j*128+dh] packed, both heads interleaved
        vsb = qkv.tile([128, (S // 128) * DOUT], BF, tag="vsb", name="vsb")
        oT = [qkv.tile([128, S], BF, tag=f"oT{j}", name=f"oT{j}") for j in range(HPC)]

        with tc.tile_pool(name="psm", bufs=1, space="PSUM") as psm:
            # PSUM banks: pqk 2 (projections + deferred out-proj pf),
            # pscr 2x2 (attention score pairs), po 1, pd 1 = 8 banks.

            def emit_outproj_block(prev, mb):
                """Out-proj m-block mb (0..3) of the previous query tile.
                8 matmuls + 4 evacuations + 1 row DMA; interleaved into the
                attention phase to fill exp-paced PE gaps."""
                if prev is None:
                    return
                pb, pt = prev
                m = 4 * pt + mb
                ob = opool.tile([128, D], BF, tag="ob", bufs=2, name="ob")
                for n in range(D // 512):
                    pf = psm.tile([128, 512], F32, tag="pqk", bufs=2, name="pf")
                    for jj in range(HPC):
                        nc.tensor.matmul(
                            pf[:], oT[jj][:, m * 128:(m + 1) * 128],
                            wo_sb[:, jj * D + n * 512: jj * D + (n + 1) * 512],
                            start=jj == 0, stop=jj == HPC - 1)
                    if (m + n) % 2 == 0:
                        nc.vector.tensor_copy(ob[:, n * 512:(n + 1) * 512], pf[:])
                    else:
                        nc.scalar.copy(ob[:, n * 512:(n + 1) * 512], pf[:])
                nc.sync.dma_start(
                    outp[pb * S + m * 128: pb * S + (m + 1) * 128, :], ob[:])

            prev = None
            for b in range(B):
                for t in range(NSQ):
                    s0 = t * SQ
                    bt = b * NSQ + t
                    # --- x tiles: contiguous [128, 4096] DMAs -----------
                    xbt = [xpool.tile([128, XTILE], BF, tag="xb", bufs=4,
                                      name=f"xbt{hh}") for hh in range(2)]
                    for hh in range(2):
                        src = xP[:, (hh * B * NSQ + bt) * XTILE:
                                 (hh * B * NSQ + bt + 1) * XTILE]
                        if bt == 0:
                            # split first tile's DMAs so compute starts early
                            nc.sync.dma_start(xbt[hh][:, 0:XTILE // 2],
                                              src[:, 0:XTILE // 2])
                            if hh == 0:
                                nc.sync.dma_start(
                                    wq_sb[:, 0:KM * DOUT // 2],
                                    wq[:, 0:KM * DOUT // 2])
                            nc.sync.dma_start(xbt[hh][:, XTILE // 2:],
                                              src[:, XTILE // 2:])
                            if hh == 0:
                                nc.sync.dma_start(
                                    wq_sb[:, KM * DOUT // 2:],
                                    wq[:, KM * DOUT // 2:])
                        else:
                            nc.sync.dma_start(xbt[hh][:], src)
                    if bt == 0:
                        nc.sync.dma_start(wk_sb[:], wk[:])
                        nc.sync.dma_start(wv_sb[:], wv[:])
                        nc.sync.dma_start(cos_sb[:], cosf[:])
                        nc.sync.dma_start(sin_sb[:], sins[:])
                        nc.sync.dma_start(mask_sb[:], masks[:])
                        nc.sync.dma_start(wo_sb[:], wo[:])

                    # --- q/k projections + RoPE, head by head -----------
                    for j in range(HPC):
                        for w_sb, dstt in ((wq_sb, qT[j]), (wk_sb, kT[j])):
                            pp = psm.tile([128, SQ], F32, tag="pqk", bufs=2,
                                          name="pp")
                            for km in range(KM):
                                nc.tensor.matmul(
                                    pp[:],
                                    w_sb[:, km * DOUT + j * DH:
                                         km * DOUT + (j + 1) * DH],
                                    xbt[km // 8][:, (km % 8) * SQ:
                                                 (km % 8 + 1) * SQ],
                                    start=km == 0, stop=km == KM - 1)
                            ppb = rope.tile([128, SQ], BF, tag="ppb", bufs=3,
                                            name="ppb")
                            nc.scalar.copy(ppb[:], pp[:])
                            rt = rope.tile([128, SQ], BF, tag="rot", bufs=2,
                                           name="rt")
                            # sin_sb rows 0:64 = +sin, rows 64:128 = -sin so
                            # both SBUF inputs share a base partition.
                            nc.vector.tensor_mul(
                                rt[0:HALF, :], ppb[HALF:128, :],
                                sin_sb[HALF:128, s0:s0 + SQ])
                            nc.vector.tensor_mul(
                                rt[HALF:128, :], ppb[0:HALF, :],
                                sin_sb[0:HALF, s0:s0 + SQ])
                            m1 = rope.tile([128, SQ], BF, tag="m1", bufs=2,
                                           name="m1")
                            nc.vector.tensor_mul(m1[:], ppb[:],
                                                 cos_sb[:, s0:s0 + SQ])
                            nc.vector.tensor_add(dstt[:, s0:s0 + SQ],
                                                 m1[:], rt[:])

                    # --- V projection directly in [seq, dh] layout ------
                    # stationary = x tile slice, moving = wv -> out rows are
                    # sequence positions; no transpose needed.
                    for sb in range(4):
                        pv = psm.tile([128, DOUT], F32, tag="pqk", bufs=2,
                                      name="pv")
                        for km in range(KM):
                            nc.tensor.matmul(
                                pv[:],
                                xbt[km // 8][:, (km % 8) * SQ + sb * 128:
                                             (km % 8) * SQ + (sb + 1) * 128],
                                wv_sb[:, km * DOUT:(km + 1) * DOUT],
                                start=km == 0, stop=km == KM - 1)
                        u = 4 * t + sb
                        if sb % 2 == 0:
                            nc.vector.tensor_copy(
                                vsb[:, u * DOUT:(u + 1) * DOUT], pv[:])
                        else:
                            nc.scalar.copy(
                                vsb[:, u * DOUT:(u + 1) * DOUT], pv[:])

                    # --- causal attention for this query tile -----------
                    for j in range(HPC):
                        nblk = 4 * t + 4
                        npair = nblk // 2
                        po = psm.tile([128, SQ], F32, tag="po", name="po")
                        pd = psm.tile([128, SQ], F32, tag="pd", name="pd")
                        prev_et = None
                        qs2s = []
                        for p in range(npair):
                            pscr = psm.tile([128, 2 * SQ], F32, tag="pscr",
                                            bufs=2, name="pscr")
                            for h in range(2):
                                u = 2 * p + h
                                nc.tensor.matmul(
                                    pscr[:, h * SQ:(h + 1) * SQ],
                                    kT[j][:, u * SKB:(u + 1) * SKB],
                                    qT[j][:, s0:s0 + SQ], start=True, stop=True,
                                    skip_group_check=True)
                            et = attn.tile([128, 2 * SQ], BF, tag="et", bufs=4,
                                           name="et")
                            nc.scalar.activation(
                                et[:], pscr[:], mybir.ActivationFunctionType.Exp,
                                scale=SCALE)
                            if 2 * p >= 4 * t:  # pair on the diagonal band
                                r = 2 * p - 4 * t   # 0 or 2
                                nc.vector.tensor_mul(
                                    et[:], et[:],
                                    mask_sb[:, r * SQ:(r + 2) * SQ])
                            for h in range(2):
                                u = 2 * p + h
                                off = (u - 4 * t) * SKB if u >= 4 * t else 0
                                nc.tensor.matmul(
                                    po[:, off:SQ],
                                    vsb[:, u * DOUT + j * DH:
                                        u * DOUT + (j + 1) * DH],
                                    et[:, h * SQ + off:(h + 1) * SQ],
                                    start=u == 0, stop=u == nblk - 1,
                                    skip_group_check=True)
                            if p % 2 == 1:
                                qs = attn.tile([128, 2 * SQ], BF, tag="qs",
                                               bufs=2, name="qs")
                                nc.vector.tensor_add(qs[:], prev_et[:], et[:])
                                qs2 = attn.tile([128, SQ], BF, tag="qs2",
                                                bufs=8, name="qs2")
                                nc.vector.tensor_add(
                                    qs2[:], qs[:, 0:SQ], qs[:, SQ:2 * SQ])
                                qs2s.append(qs2)
                            prev_et = et
                            # fill exp-paced gaps with deferred out-proj
                            if p == 0:
                                emit_outproj_block(prev, 2 * j)
                        # denominator accumulation, then normalize
                        nquad = len(qs2s)
                        for qi, q2 in enumerate(qs2s):
                            nc.tensor.matmul(
                                pd[:], ones_sb[:], q2[:],
                                start=qi == 0, stop=qi == nquad - 1)
                        emit_outproj_block(prev, 2 * j + 1)
                        rec = attn.tile([128, SQ], F32, tag="rec", bufs=2,
                                        name="rec")
                        nc.vector.reciprocal_approx_fast(rec[:], pd[:])
                        nc.vector.tensor_mul(oT[j][:, s0:s0 + SQ], po[:], rec[:])
                    prev = (b, t)
            # final tile's out-proj
            for mb in range(4):
                emit_outproj_block(prev, mb)

    nc.compile()
    return nc


def _host_inputs(x, wq, wk, wv, wo, cos, sin):
    bf16 = ml_dtypes.bfloat16
    # xP[p, hh, bt, a, n] = x[b, t*512+n, hh*1024 + a*128 + p]
    xb = np.ascontiguousarray(
        x.reshape(B * S, D).T).astype(bf16)           # [D, B*S]
    xP = np.ascontiguousarray(
        xb.reshape(2, 8, 128, B, NSQ, SQ)
        .transpose(2, 0, 3, 4, 1, 5).reshape(128, -1))

    def pack_w(w):  # [D, 256] -> [128, km*256+n]
        return np.ascontiguousarray(
            w.reshape(KM, 128, DOUT).transpose(1, 0, 2).reshape(128, -1)
        ).astype(bf16)

    cos = np.asarray(cos, dtype=np.float32)        # [S, 64]
    sin = np.asarray(sin, dtype=np.float32)
    cosf = np.ascontiguousarray(
        np.concatenate([cos, cos], axis=1).T).astype(bf16)   # [128, S]
    sinf = np.concatenate([sin, -sin], axis=1).T   # rows 64-127 negated
    sinf = np.ascontiguousarray(sinf).astype(bf16)

    i = np.arange(SKB)[:, None]
    jj = np.arange(SQ)[None, :]
    masks = np.concatenate(
        [(i + r * SKB <= jj) for r in range(4)], axis=1).astype(bf16)
    ones_h = np.ones((128, 128), dtype=bf16)

    in_maps = []
    for c in range(NC):
        lo = c * DOUT
        wop = np.ascontiguousarray(
            wo[lo:lo + DOUT, :].reshape(HPC, 128, D)
            .transpose(1, 0, 2).reshape(128, -1)).astype(bf16)
        in_maps.append({
            "xP": xP,
            "wq": pack_w(np.ascontiguousarray(wq[:, lo:lo + DOUT])),
            "wk": pack_w(np.ascontiguousarray(wk[:, lo:lo + DOUT])),
            "wv": pack_w(np.ascontiguousarray(wv[:, lo:lo + DOUT])),
            "wo": wop,
            "cosf": cosf,
            "sins": sinf,
            "masks": masks,
            "ones": ones_h,
        })
    return in_maps


def kernel(x, wq, wk, wv, wo, cos, sin, _trace=False, _tmpdir=None):
    if "nc" not in _CACHED:
        _CACHED["nc"] = _build()
    nc = _CACHED["nc"]
    in_maps = _host_inputs(
        np.asarray(x, dtype=np.float32), np.asarray(wq, dtype=np.float32),
        np.asarray(wk, dtype=np.float32), np.asarray(wv, dtype=np.float32),
        np.asarray(wo, dtype=np.float32), cos, sin)
    res = bass_utils.run_bass_kernel_spmd(
        nc, in_maps, core_ids=list(range(NC)), trace=_trace, tmpdir=_tmpdir)
    acc = np.zeros((B * S, D), dtype=np.float32)
    for c in range(NC):
        acc += res.results[c]["outp"].astype(np.float32)
    out = acc.reshape(B, S, D)
    if _trace:
        _CACHED["last_results"] = res
    return out


# revision 10
# speedup vs baseline: 1.2749x; 1.0432x over previous
"""Dense causal transformer attention block on 8 Trainium2 NeuronCores.

Problem: out = CausalAttention(RoPE(x@wq, x@wk), x@wv) @ wo
  x [2, 4096, 2048], 16 heads x 128 dim, fp32 I/O.

Sharding: tensor-parallel over heads. Core c owns heads {2c, 2c+1}:
  - computes qT/kT ([head_dim, seq] layout) for its heads from the
    host-packed xP (all DMAs are contiguous-row), RoPE applied on-chip in
    bf16 (one ScalarE PSUM->SBUF copy, then 2x-mode DVE ops),
  - V is projected directly in [seq, head_dim] layout by using the x tile
    as the matmul stationary operand (no PE transposes needed),
  - runs causal attention in transposed form (scoresT = k @ qT so the
    softmax weights come out as the moving operand of the A@V matmul),
    with partial-width A@V matmuls on the diagonal band,
  - denominators via an all-ones [128,128] stationary matmul on
    DVE-pre-summed exp tiles; reciprocal via the fast custom DVE op,
  - output projection for query tile t is deferred into tile t+1's
    attention phase (interleaved m-blocks) so the softmax-normalize chain
    never stalls the in-order PE queue.
Host sums the 8 partial outputs (the wo row-parallel all-reduce).

Compute dtype bf16 (PE 1 col/cycle), accumulation fp32 in PSUM.
"""
import sys

for _p in ("/opt/trn_rl_repo",):
    if _p not in sys.path:
        sys.path.insert(0, _p)

import numpy as np
import ml_dtypes
from contextlib import ExitStack

import concourse.bass as bass
import concourse.tile as tile
from concourse import bacc, mybir
from concourse import bass_utils

B, S, D = 2, 4096, 2048
H, DH = 16, 128
HALF = DH // 2
NC = 8
HPC = H // NC          # heads per core = 2
DOUT = HPC * DH        # 256 local proj width
ROPE_BASE = 10000.0
SCALE = 1.0 / float(np.sqrt(DH))
SQ = 512               # query tile (free dim of scoresT)
SKB = 128              # key block (partitions of scoresT)
KM = D // 128          # 16 contraction blocks
NSQ = S // SQ          # 8 query tiles per batch
BF = mybir.dt.bfloat16
F32 = mybir.dt.float32

_CACHED = {}


def _build():
    nc = bacc.Bacc("TRN2", target_bir_lowering=False, debug=False, num_devices=NC)

    # xP: [128, (hh, b*8+t, a*512+n)] so each (b,t,hh) x-tile DMA is a
    # contiguous [128, 4096] read. Weights pre-packed the same way.
    xP = nc.dram_tensor("xP", [128, 2 * B * NSQ * 8 * SQ], BF,
                        kind="ExternalInput").ap()
    wq = nc.dram_tensor("wq", [128, KM * DOUT], BF, kind="ExternalInput").ap()
    wk = nc.dram_tensor("wk", [128, KM * DOUT], BF, kind="ExternalInput").ap()
    wv = nc.dram_tensor("wv", [128, KM * DOUT], BF, kind="ExternalInput").ap()
    wo = nc.dram_tensor("wo", [128, HPC * D], BF, kind="ExternalInput").ap()
    cosf = nc.dram_tensor("cosf", [DH, S], BF, kind="ExternalInput").ap()
    sins = nc.dram_tensor("sins", [DH, S], BF, kind="ExternalInput").ap()
    masks = nc.dram_tensor("masks", [SKB, 4 * SQ], BF, kind="ExternalInput").ap()
    ones = nc.dram_tensor("ones", [128, 128], BF, kind="ExternalInput").ap()
    outp = nc.dram_tensor("outp", [B * S, D], BF, kind="ExternalOutput").ap()

    XTILE = 8 * SQ     # 4096 cols per (b,t,hh) x tile

    with tile.TileContext(nc) as tc, ExitStack() as ctx:
        const = ctx.enter_context(tc.tile_pool(name="const", bufs=1))
        xpool = ctx.enter_context(tc.tile_pool(name="xpool", bufs=1))
        qkv = ctx.enter_context(tc.tile_pool(name="qkv", bufs=1))
        rope = ctx.enter_context(tc.tile_pool(name="rope", bufs=2))
        attn = ctx.enter_context(tc.tile_pool(name="attn", bufs=4))
        opool = ctx.enter_context(tc.tile_pool(name="opool", bufs=2))

        # ---- persistent constants (DMAs emitted in priority order) -----
        # wq/wk split in halves (km 0-7 / 8-15) so the first projection
        # matmuls start as soon as ~0.5 MB has landed.
        wq_sb = [const.tile([128, KM * DOUT // 2], BF, name=f"wq_sb{i}")
                 for i in range(2)]
        wk_sb = [const.tile([128, KM * DOUT // 2], BF, name=f"wk_sb{i}")
                 for i in range(2)]
        wv_sb = const.tile([128, KM * DOUT], BF, name="wv_sb")
        ones_sb = const.tile([128, 128], BF, name="ones_sb")
        cos_sb = const.tile([DH, S], BF, name="cos_sb")
        sin_sb = const.tile([DH, S], BF, name="sin_sb")  # rows 64-127 = -sin
        mask_sb = const.tile([SKB, 4 * SQ], BF, name="mask_sb")
        wo_sb = const.tile([128, HPC * D], BF, name="wo_sb")   # [p, jj*2048+n]

        qT = [qkv.tile([128, S], BF, tag=f"qT{j}", name=f"qT{j}") for j in range(HPC)]
        kT = [qkv.tile([128, S], BF, tag=f"kT{j}", name=f"kT{j}") for j in range(HPC)]
        # vsb: [seq-block u][j*128+dh] packed, both heads interleaved
        vsb = qkv.tile([128, (S // 128) * DOUT], BF, tag="vsb", name="vsb")
        oT = [qkv.tile([128, S], BF, tag=f"oT{j}", name=f"oT{j}") for j in range(HPC)]

        with tc.tile_pool(name="psm", bufs=1, space="PSUM") as psm:
            # PSUM banks: pqk 2 (projections + deferred out-proj pf),
            # pscr 2x2 (attention score pairs), po 1, pd 1 = 8 banks.

            def emit_outproj_block(prev, mb, tags=("pqk",)):
                """Out-proj m-block mb (0..3) of the previous query tile.
                8 matmuls + 4 evacuations + 1 row DMA; interleaved into the
                attention phase to fill exp-paced PE gaps."""
                if prev is None:
                    return
                pb, pt = prev
                m = 4 * pt + mb
                ob = opool.tile([128, D], BF, tag="ob", bufs=2, name="ob")
                for n in range(D // 512):
                    pf = psm.tile([128, 512], F32, tag=tags[n % len(tags)],
                                  bufs=2, name="pf")
                    for jj in range(HPC):
                        nc.tensor.matmul(
                            pf[:], oT[jj][:, m * 128:(m + 1) * 128],
                            wo_sb[:, jj * D + n * 512: jj * D + (n + 1) * 512],
                            start=jj == 0, stop=jj == HPC - 1)
                    if (m + n) % 2 == 0:
                        nc.vector.tensor_copy(ob[:, n * 512:(n + 1) * 512], pf[:])
                    else:
                        nc.scalar.copy(ob[:, n * 512:(n + 1) * 512], pf[:])
                nc.sync.dma_start(
                    outp[pb * S + m * 128: pb * S + (m + 1) * 128, :], ob[:])

            prev = None
            for b in range(B):
                for t in range(NSQ):
                    s0 = t * SQ
                    bt = b * NSQ + t
                    # --- x tile: 4 quarter tiles, contiguous DMAs -------
                    # quarter q holds contraction blocks km = 4q..4q+3.
                    xbt = [xpool.tile([128, XTILE // 2], BF, tag="xb", bufs=8,
                                      name=f"xbt{qq}") for qq in range(4)]
                    xsrc = [xP[:, ((qq // 2) * B * NSQ + bt) * XTILE
                               + (qq % 2) * (XTILE // 2):
                               ((qq // 2) * B * NSQ + bt) * XTILE
                               + (qq % 2 + 1) * (XTILE // 2)]
                            for qq in range(4)]
                    if bt == 0:
                        # interleave x quarters with weight halves so the
                        # first matmuls start after ~1 MB of DMA.
                        nc.sync.dma_start(xbt[0][:], xsrc[0])
                        nc.sync.dma_start(wq_sb[0][:], wq[:, 0:KM * DOUT // 2])
                        nc.sync.dma_start(xbt[1][:], xsrc[1])
                        nc.sync.dma_start(wq_sb[1][:], wq[:, KM * DOUT // 2:])
                        nc.sync.dma_start(xbt[2][:], xsrc[2])
                        nc.sync.dma_start(wk_sb[0][:], wk[:, 0:KM * DOUT // 2])
                        nc.sync.dma_start(xbt[3][:], xsrc[3])
                        nc.sync.dma_start(wk_sb[1][:], wk[:, KM * DOUT // 2:])
                        nc.sync.dma_start(wv_sb[:], wv[:])
                        nc.sync.dma_start(cos_sb[:], cosf[:])
                        nc.sync.dma_start(sin_sb[:], sins[:])
                        nc.sync.dma_start(mask_sb[:], masks[:])
                        nc.sync.dma_start(wo_sb[:], wo[:])
                        nc.sync.dma_start(ones_sb[:], ones[:])
                    else:
                        for qq in range(4):
                            nc.sync.dma_start(xbt[qq][:], xsrc[qq])

                    def xsl(km, c0, c1):
                        return xbt[km // 4][:, (km % 4) * SQ + c0:
                                            (km % 4) * SQ + c1]

                    # --- q/k projections + RoPE, head by head -----------
                    for j in range(HPC):
                        for w_sb, dstt in ((wq_sb, qT[j]), (wk_sb, kT[j])):
                            pp = psm.tile([128, SQ], F32, tag="pqk", bufs=2,
                                          name="pp")
                            for km in range(KM):
                                nc.tensor.matmul(
                                    pp[:],
                                    w_sb[km // 8][
                                        :, (km % 8) * DOUT + j * DH:
                                        (km % 8) * DOUT + (j + 1) * DH],
                                    xsl(km, 0, SQ),
                                    start=km == 0, stop=km == KM - 1)
                            ppb = rope.tile([128, SQ], BF, tag="ppb", bufs=3,
                                            name="ppb")
                            nc.scalar.copy(ppb[:], pp[:])
                            rt = rope.tile([128, SQ], BF, tag="rot", bufs=2,
                                           name="rt")
                            # sin_sb rows 0:64 = +sin, rows 64:128 = -sin so
                            # both SBUF inputs share a base partition.
                            nc.vector.tensor_mul(
                                rt[0:HALF, :], ppb[HALF:128, :],
                                sin_sb[HALF:128, s0:s0 + SQ])
                            nc.vector.tensor_mul(
                                rt[HALF:128, :], ppb[0:HALF, :],
                                sin_sb[0:HALF, s0:s0 + SQ])
                            m1 = rope.tile([128, SQ], BF, tag="m1", bufs=2,
                                           name="m1")
                            nc.vector.tensor_mul(m1[:], ppb[:],
                                                 cos_sb[:, s0:s0 + SQ])
                            nc.vector.tensor_add(dstt[:, s0:s0 + SQ],
                                                 m1[:], rt[:])

                    # --- V projection directly in [seq, dh] layout ------
                    # stationary = x tile slice, moving = wv -> out rows are
                    # sequence positions; no transpose needed.
                    for sb in range(4):
                        pv = psm.tile([128, DOUT], F32, tag="pqk", bufs=2,
                                      name="pv")
                        for km in range(KM):
                            nc.tensor.matmul(
                                pv[:],
                                xsl(km, sb * 128, (sb + 1) * 128),
                                wv_sb[:, km * DOUT:(km + 1) * DOUT],
                                start=km == 0, stop=km == KM - 1)
                        u = 4 * t + sb
                        if sb % 2 == 0:
                            nc.vector.tensor_copy(
                                vsb[:, u * DOUT:(u + 1) * DOUT], pv[:])
                        else:
                            nc.scalar.copy(
                                vsb[:, u * DOUT:(u + 1) * DOUT], pv[:])

                    # --- causal attention for this query tile -----------
                    # All 4 deferred out-proj blocks are emitted inside the
                    # j=0 phase (before any oT write of this tile) so they
                    # never wait on this tile's normalize chain.
                    for j in range(HPC):
                        nblk = 4 * t + 4
                        npair = nblk // 2
                        po = psm.tile([128, SQ], F32, tag="po", name="po")
                        pd = psm.tile([128, SQ], F32, tag="pd", name="pd")
                        prev_et = None
                        qs2s = []
                        for p in range(npair):
                            pscr = psm.tile([128, 2 * SQ], F32, tag="pscr",
                                            bufs=2, name="pscr")
                            diag = 2 * p >= 4 * t
                            for h in range(2):
                                u = 2 * p + h
                                off = (u - 4 * t) * SKB if (diag and bt) else 0
                                nc.tensor.matmul(
                                    pscr[:, h * SQ + off:(h + 1) * SQ],
                                    kT[j][:, u * SKB:(u + 1) * SKB],
                                    qT[j][:, s0 + off:s0 + SQ],
                                    start=True, stop=True,
                                    skip_group_check=True)
                            et = attn.tile([128, 2 * SQ], BF, tag="et", bufs=4,
                                           name="et")
                            if diag and bt and 2 * p - 4 * t == 2:
                                # steep diagonal pair: exp only the live
                                # regions [256:512] and [896:1024]
                                nc.scalar.activation(
                                    et[:, 256:512], pscr[:, 256:512],
                                    mybir.ActivationFunctionType.Exp,
                                    scale=SCALE)
                                nc.scalar.activation(
                                    et[:, 896:1024], pscr[:, 896:1024],
                                    mybir.ActivationFunctionType.Exp,
                                    scale=SCALE)
                            else:
                                nc.scalar.activation(
                                    et[:], pscr[:],
                                    mybir.ActivationFunctionType.Exp,
                                    scale=SCALE)
                            if diag:  # mask also zeroes any stale region
                                r = 2 * p - 4 * t   # 0 or 2
                                nc.vector.tensor_mul(
                                    et[:], et[:],
                                    mask_sb[:, r * SQ:(r + 2) * SQ])
                            for h in range(2):
                                u = 2 * p + h
                                off = (u - 4 * t) * SKB if diag else 0
                                nc.tensor.matmul(
                                    po[:, off:SQ],
                                    vsb[:, u * DOUT + j * DH:
                                        u * DOUT + (j + 1) * DH],
                                    et[:, h * SQ + off:(h + 1) * SQ],
                                    start=u == 0, stop=u == nblk - 1,
                                    skip_group_check=True)
                            if p % 2 == 1:
                                qs = attn.tile([128, 2 * SQ], BF, tag="qs",
                                               bufs=2, name="qs")
                                nc.vector.tensor_add(qs[:], prev_et[:], et[:])
                                qs2 = attn.tile([128, SQ], BF, tag="qs2",
                                                bufs=8, name="qs2")
                                nc.vector.tensor_add(
                                    qs2[:], qs[:, 0:SQ], qs[:, SQ:2 * SQ])
                                qs2s.append(qs2)
                            prev_et = et
                            # fill exp-paced gaps with deferred out-proj
                            if j == 0 and p in (0, 1):
                                emit_outproj_block(prev, p)
                        # pre-sum qs2 pairs on DVE to halve the ones-matmuls
                        dsum = []
                        for qi in range(0, len(qs2s) - 1, 2):
                            q4 = attn.tile([128, SQ], BF, tag="q4", bufs=4,
                                           name="q4")
                            nc.vector.tensor_add(
                                q4[:], qs2s[qi][:], qs2s[qi + 1][:])
                            dsum.append(q4)
                        if len(qs2s) % 2:
                            dsum.append(qs2s[-1])
                        for qi, q2 in enumerate(dsum):
                            nc.tensor.matmul(
                                pd[:], ones_sb[:], q2[:],
                                start=qi == 0, stop=qi == len(dsum) - 1)
                        if j == 0:
                            emit_outproj_block(prev, 2)
                        rec = attn.tile([128, SQ], F32, tag="rec", bufs=2,
                                        name="rec")
                        nc.vector.reciprocal_approx_fast(rec[:], pd[:])
                        if j == 0:
                            emit_outproj_block(prev, 3)
                        nc.vector.tensor_mul(oT[j][:, s0:s0 + SQ], po[:], rec[:])
                    prev = (b, t)
            # final tile's out-proj: alternate PSUM tags for deeper pipeline
            for mb in range(4):
                emit_outproj_block(prev, mb, tags=("pqk", "pscr"))

    nc.compile()
    return nc


def _host_inputs(x, wq, wk, wv, wo, cos, sin):
    bf16 = ml_dtypes.bfloat16
    # xP[p, hh, bt, a, n] = x[b, t*512+n, hh*1024 + a*128 + p]
    xb = np.ascontiguousarray(
        x.reshape(B * S, D).T).astype(bf16)           # [D, B*S]
    xP = np.ascontiguousarray(
        xb.reshape(2, 8, 128, B, NSQ, SQ)
        .transpose(2, 0, 3, 4, 1, 5).reshape(128, -1))

    def pack_w(w):  # [D, 256] -> [128, km*256+n]
        return np.ascontiguousarray(
            w.reshape(KM, 128, DOUT).transpose(1, 0, 2).reshape(128, -1)
        ).astype(bf16)

    cos = np.asarray(cos, dtype=np.float32)        # [S, 64]
    sin = np.asarray(sin, dtype=np.float32)
    cosf = np.ascontiguousarray(
        np.concatenate([cos, cos], axis=1).T).astype(bf16)   # [128, S]
    sinf = np.concatenate([sin, -sin], axis=1).T   # rows 64-127 negated
    sinf = np.ascontiguousarray(sinf).astype(bf16)

    i = np.arange(SKB)[:, None]
    jj = np.arange(SQ)[None, :]
    masks = np.concatenate(
        [(i + r * SKB <= jj) for r in range(4)], axis=1).astype(bf16)
    ones_h = np.ones((128, 128), dtype=bf16)

    in_maps = []
    for c in range(NC):
        lo = c * DOUT
        wop = np.ascontiguousarray(
            wo[lo:lo + DOUT, :].reshape(HPC, 128, D)
            .transpose(1, 0, 2).reshape(128, -1)).astype(bf16)
        in_maps.append({
            "xP": xP,
            "wq": pack_w(np.ascontiguousarray(wq[:, lo:lo + DOUT])),
            "wk": pack_w(np.ascontiguousarray(wk[:, lo:lo + DOUT])),
            "wv": pack_w(np.ascontiguousarray(wv[:, lo:lo + DOUT])),
            "wo": wop,
            "cosf": cosf,
            "sins": sinf,
            "masks": masks,
            "ones": ones_h,
        })
    return in_maps


def kernel(x, wq, wk, wv, wo, cos, sin, _trace=False, _tmpdir=None):
    if "nc" not in _CACHED:
        _CACHED["nc"] = _build()
    nc = _CACHED["nc"]
    in_maps = _host_inputs(
        np.asarray(x, dtype=np.float32), np.asarray(wq, dtype=np.float32),
        np.asarray(wk, dtype=np.float32), np.asarray(wv, dtype=np.float32),
        np.asarray(wo, dtype=np.float32), cos, sin)
    res = bass_utils.run_bass_kernel_spmd(
        nc, in_maps, core_ids=list(range(NC)), trace=_trace, tmpdir=_tmpdir)
    acc = np.zeros((B * S, D), dtype=np.float32)
    for c in range(NC):
        acc += res.results[c]["outp"].astype(np.float32)
    out = acc.reshape(B, S, D)
    if _trace:
        _CACHED["last_results"] = res
    return out
